# revision 1
# baseline (speedup 1.0000x reference)
"""Data-parallel Trainium kernel for nn_CSGCNet (criss-cross attention block).

Sharding: pure data parallel over batch B=8 across the 8 NeuronCores (one
sample per core). All conv/attention math is per-sample; the BatchNorm1d
training-mode batch statistics (mean/var over (B, L)) are the only cross-core
coupling and are handled with an in-graph all-reduce (lax.pmean), so the whole
thing is a single SPMD launch on 8 cores.
"""

import functools

import jax
import jax.numpy as jnp
import numpy as np

EPS = 1e-5
GROUPS = 4
B, C, H, W = 8, 64, 160, 160


def _gconv1x1(x, w, b, groups):
    # x: [C,H,W]; w: [Cout, Cin//groups]; grouped 1x1 conv (single sample)
    Cin, Hh, Ww = x.shape
    Co = w.shape[0]
    xg = x.reshape(groups, Cin // groups, Hh, Ww)
    wg = w.reshape(groups, Co // groups, Cin // groups)
    y = jnp.einsum('gchw,goc->gohw', xg, wg).reshape(Co, Hh, Ww)
    if b is not None:
        y = y + b[:, None, None]
    return y


def _per_sample(x, wq, bq, wk, bk, wv, bv, gamma, w1d, bn_w, bn_b):
    # x: [C,H,W] one sample on one core
    q = _gconv1x1(x, wq, bq, GROUPS)   # [Cq,H,W]
    k = _gconv1x1(x, wk, bk, GROUPS)   # [Cq,H,W]
    v = _gconv1x1(x, wv, bv, GROUPS)   # [C,H,W]

    energy_H = jnp.einsum('chw,cjw->hwj', q, k)  # [H,W,H]
    diag = jnp.where(jnp.eye(H, dtype=bool), -jnp.inf, 0.0).astype(x.dtype)
    energy_H = energy_H + diag[:, None, :]
    energy_W = jnp.einsum('chw,chj->hwj', q, k)  # [H,W,W]

    att = jax.nn.softmax(jnp.concatenate([energy_H, energy_W], axis=2), axis=2)
    att_H = att[..., :H]
    att_W = att[..., H:]

    out_H = jnp.einsum('cjw,hwj->chw', v, att_H)
    out_W = jnp.einsum('chj,hwj->chw', v, att_W)
    out = gamma[0] * (out_H + out_W)             # [C,H,W]

    L = H * W
    o = out.reshape(GROUPS, C // GROUPS, L)
    w1g = w1d.reshape(GROUPS, C // GROUPS, C // GROUPS)
    o = jnp.einsum('gcl,goc->gol', o, w1g).reshape(C, L)

    # BatchNorm1d training-mode stats over (B, L): all-reduce across cores.
    mean = jax.lax.pmean(jnp.mean(o, axis=1), axis_name='b')            # [C]
    mean_sq = jax.lax.pmean(jnp.mean(o * o, axis=1), axis_name='b')     # [C]
    var = mean_sq - mean * mean
    o = (o - mean[:, None]) * jax.lax.rsqrt(var[:, None] + EPS)
    o = o * bn_w[:, None] + bn_b[:, None]

    o = o.reshape(C, H, W)
    return jax.nn.relu(o + x)


@functools.partial(
    jax.pmap, axis_name='b',
    in_axes=(0,) + (None,) * 10,
)
def _kernel_pmap(x, wq, bq, wk, bk, wv, bv, gamma, w1d, bn_w, bn_b):
    return _per_sample(x, wq, bq, wk, bk, wv, bv, gamma, w1d, bn_w, bn_b)


def kernel(x, wq, bq, wk, bk, wv, bv, gamma, w1d, bn_w, bn_b):
    out = _kernel_pmap(
        jnp.asarray(x, jnp.float32),
        jnp.asarray(wq), jnp.asarray(bq),
        jnp.asarray(wk), jnp.asarray(bk),
        jnp.asarray(wv), jnp.asarray(bv),
        jnp.asarray(gamma), jnp.asarray(w1d),
        jnp.asarray(bn_w), jnp.asarray(bn_b),
    )
    return np.asarray(out, dtype=np.float32)

